# revision 3
# baseline (speedup 1.0000x reference)
"""Trainium2 Bass kernel for nn_Decoder (2-layer LSTM decoder, B=2048,
T=96, H=1024), data-parallel over 8 NeuronCores (256 batch per core).

Self-contained: host-side prep (embedding gathers, x assembly, weight
transpose/permutation) in numpy, then one SPMD Bass/Tile kernel per
core computing the full 96-step recurrence on-device.

Per-core layout:
  - hidden/gate dim on SBUF partitions, batch (256) on the matmul free
    dim; h states kept transposed as 8 x [128, 256] bf16 chunks
    (ping-pong A/B per step); c states fp32, updated in place.
  - weights bf16, host-permuted "unit-major": unit j of a layer owns
    gate columns [i_j | f_j | g_j | o_j]; per (layer, unit) the gates
    accumulate into one [128, 1024] PSUM tile (2 banks) over K chunks.
  - ACT applies sigmoid/tanh with the per-partition bias fused; DVE
    combines into c (fp32) and h (bf16).
  - W1 hh-half resident in SBUF; W0 and W1 ih-half streamed per step
    (~1MB DMA per unit) - PE rereads weights from SBUF via LDWEIGHTS.
  - x K-dim zero-padded 60->128 so FWL (fast weight load) engages.
  - PSUM start=True clears has_written for a whole bank, so only the
    first matmul touching each bank carries it (L1 interleaves gates).
"""

import numpy as np
import ml_dtypes

import concourse.bass as bass
import concourse.mybir as mybir
import concourse.tile as tile

F32 = mybir.dt.float32
BF16 = mybir.dt.bfloat16
AF = mybir.ActivationFunctionType

B, T, FEAT = 2048, 96, 32
H = 1024
KX = 60
KXP = 128
NH = 8
NCORES = 8
BS = B // NCORES

# ---------------------------------------------------------------------
# Workaround 1: this walrus build allows only ONE sync-wait per
# instruction. Split extra waits onto same-engine NOPs at the BIR JSON
# boundary (engines execute block instructions in order, so semantics
# are preserved).
# ---------------------------------------------------------------------
import orjson
import concourse.bass2jax as bass2jax
import concourse.bass_utils as bass_utils

_orig_compile = bass_utils.compile_bir_kernel


def _split_waits(bir_bytes):
    d = orjson.loads(bir_bytes)
    nop_id = 0
    for fn in d.get("functions", []):
        for bb in fn.get("blocks", []):
            out = []
            for inst in bb["instructions"]:
                si = inst.get("sync_info")
                waits = (si or {}).get("on_wait") or []
                if len(waits) > 1:
                    for w in waits[:-1]:
                        nop_id += 1
                        out.append({
                            "debug": inst.get("debug", 0),
                            "engine": inst["engine"],
                            "ins": [],
                            "name": f"{inst['name']}-wsplit{nop_id}",
                            "opcode": "NoOp",
                            "outs": [],
                            "sync_info": {"on_update": [], "on_wait": [w]},
                            "text_hint": "wait_split",
                        })
                    si["on_wait"] = waits[-1:]
                out.append(inst)
            bb["instructions"] = out
    return orjson.dumps(d)


def _patched_compile(ant_bir_str, *args, **kwargs):
    try:
        ant_bir_str = _split_waits(ant_bir_str)
    except Exception as e:
        print(f"kernel.py: wait-split failed ({e!r}); compiling unpatched")
    return _orig_compile(ant_bir_str, *args, **kwargs)


if bass_utils.compile_bir_kernel.__name__ != "_patched_compile":
    bass_utils.compile_bir_kernel = _patched_compile
    bass2jax.compile_bir_kernel = _patched_compile


# ---------------------------------------------------------------------
# Kernel emission
# ---------------------------------------------------------------------

def _gate_perm():
    j = np.arange(8)[:, None, None]
    g = np.arange(4)[None, :, None]
    u = np.arange(128)[None, None, :]
    return (g * H + j * 128 + u).reshape(-1)


def _emit_kernel(nc, stream_bufs=4, gate_bufs=3):
    from contextlib import ExitStack

    xt_d = nc.dram_tensor("xt", [T, KXP, BS], BF16, kind="ExternalInput").ap()
    w0x_d = nc.dram_tensor("w0x", [NH, KXP, 512], BF16, kind="ExternalInput").ap()
    w0h_d = nc.dram_tensor("w0h", [NH, NH, 128, 512], BF16, kind="ExternalInput").ap()
    w1i_d = nc.dram_tensor("w1i", [NH, NH, 128, 512], BF16, kind="ExternalInput").ap()
    w1h_d = nc.dram_tensor("w1h", [NH, NH, 128, 512], BF16, kind="ExternalInput").ap()
    bias_d = nc.dram_tensor("bias", [128, 64], F32, kind="ExternalInput").ap()
    wproj_d = nc.dram_tensor("wproj", [128, NH], BF16, kind="ExternalInput").ap()
    hinit_d = nc.dram_tensor("hinit", [2, H, BS], BF16, kind="ExternalInput").ap()
    cinit_d = nc.dram_tensor("cinit", [2, H, BS], F32, kind="ExternalInput").ap()
    y_d = nc.dram_tensor("y", [T, BS], F32, kind="ExternalOutput").ap()

    with tile.TileContext(nc) as tc, ExitStack() as ctx:
        singles = ctx.enter_context(tc.tile_pool(name="singles", bufs=1))
        w0xp = ctx.enter_context(tc.tile_pool(name="w0xp", bufs=stream_bufs))
        w0hp = ctx.enter_context(tc.tile_pool(name="w0hp", bufs=stream_bufs))
        w1ip = ctx.enter_context(tc.tile_pool(name="w1ip", bufs=stream_bufs))
        xtpool = ctx.enter_context(tc.tile_pool(name="xtpool", bufs=4))
        gpool = ctx.enter_context(
            tc.tile_pool(name="gpool", bufs=gate_bufs, space="PSUM"))
        ppool = ctx.enter_context(tc.tile_pool(name="ppool", bufs=2, space="PSUM"))
        spool = ctx.enter_context(tc.tile_pool(name="spool", bufs=3))
        tpool = ctx.enter_context(tc.tile_pool(name="tpool", bufs=4))

        w1h = []
        for j in range(NH):
            t_ = singles.tile([128, NH, 512], BF16, name=f"w1h{j}", tag=f"w1h{j}")
            nc.sync.dma_start(out=t_, in_=w1h_d[j].rearrange("k p c -> p k c"))
            w1h.append(t_)
        bias = singles.tile([128, 64], F32, name="bias", tag="bias")
        nc.sync.dma_start(out=bias, in_=bias_d)
        wproj = singles.tile([128, NH], BF16, name="wproj", tag="wproj")
        nc.sync.dma_start(out=wproj, in_=wproj_d)

        hT = [[[singles.tile([128, BS], BF16, name=f"h{p}_{l}_{k}",
                             tag=f"h{p}_{l}_{k}")
                for k in range(NH)] for l in range(2)] for p in range(2)]
        cT = [[singles.tile([128, BS], F32, name=f"c{l}_{k}", tag=f"c{l}_{k}")
               for k in range(NH)] for l in range(2)]
        for l in range(2):
            for k in range(NH):
                nc.sync.dma_start(out=hT[0][l][k],
                                  in_=hinit_d[l, 128 * k : 128 * (k + 1), :])
                nc.sync.dma_start(out=cT[l][k],
                                  in_=cinit_d[l, 128 * k : 128 * (k + 1), :])

        def bcol(layer, j, g):
            i = layer * 32 + j * 4 + g
            return bias[:, i : i + 1]

        def elementwise(layer, j, psum, h_out):
            c = cT[layer][j]
            sact = spool.tile([128, 4 * BS], F32, name="sact", tag="sact")
            for g, fn in ((0, AF.Sigmoid), (1, AF.Sigmoid),
                          (2, AF.Tanh), (3, AF.Sigmoid)):
                nc.scalar.activation(sact[:, g * BS : (g + 1) * BS],
                                     psum[:, g * BS : (g + 1) * BS],
                                     fn, bias=bcol(layer, j, g))
            tmp = tpool.tile([128, BS], F32, name="tmp", tag="tmp")
            nc.vector.tensor_mul(tmp, sact[:, 0 * BS : 1 * BS],
                                 sact[:, 2 * BS : 3 * BS])
            nc.vector.tensor_mul(c, c, sact[:, 1 * BS : 2 * BS])
            nc.vector.tensor_add(c, c, tmp)
            tct = tpool.tile([128, BS], F32, name="tct", tag="tct")
            nc.scalar.activation(tct, c, AF.Tanh)
            nc.vector.tensor_mul(h_out, sact[:, 3 * BS : 4 * BS], tct)

        for t in range(T):
            A = t % 2
            Bp = 1 - A

            xt = xtpool.tile([KXP, BS], BF16, name="xt", tag="xt")
            nc.sync.dma_start(out=xt, in_=xt_d[t])

            # layer 0: units sequential; stream W0 per unit
            for j in range(NH):
                w0x = w0xp.tile([KXP, 512], BF16, name="w0x", tag="w0x")
                nc.sync.dma_start(out=w0x, in_=w0x_d[j])
                w0h = w0hp.tile([128, NH, 512], BF16, name="w0h", tag="w0h")
                nc.sync.dma_start(out=w0h, in_=w0h_d[j].rearrange("k p c -> p k c"))
                psum = gpool.tile([128, 4 * BS], F32, name="gates", tag="gates")
                for g in range(4):
                    out = psum[:, g * BS : (g + 1) * BS]
                    gsl = slice(g * 128, (g + 1) * 128)
                    nc.tensor.matmul(out, w0x[:, gsl], xt,
                                     start=(g % 2 == 0), stop=False,
                                     skip_group_check=True)
                    for k in range(NH):
                        nc.tensor.matmul(out, w0h[:, k, gsl], hT[A][0][k],
                                         start=False, stop=(k == NH - 1),
                                         skip_group_check=True)
                elementwise(0, j, psum, hT[Bp][0][j])

            # layer 1: waves of 2 units; hh chunks first, then ih
            for w in range(4):
                units = (2 * w, 2 * w + 1)
                w1i = {}
                for j in units:
                    w1i[j] = w1ip.tile([128, NH, 512], BF16, name="w1i", tag="w1i")
                    nc.sync.dma_start(out=w1i[j],
                                      in_=w1i_d[j].rearrange("k p c -> p k c"))
                psums = {j: gpool.tile([128, 4 * BS], F32, name="gates",
                                       tag="gates") for j in units}
                for k in range(16):
                    rhs = hT[A][1][k] if k < NH else hT[Bp][0][k - NH]
                    for j in units:
                        lhsT = w1h[j] if k < NH else w1i[j]
                        kk = k if k < NH else k - NH
                        for g in range(4):
                            out = psums[j][:, g * BS : (g + 1) * BS]
                            # start=True clears has_written for the WHOLE
                            # bank; only the first matmul touching each
                            # bank may carry it.
                            st = k == 0 and g % 2 == 0
                            nc.tensor.matmul(
                                out, lhsT[:, kk, g * 128 : (g + 1) * 128],
                                rhs, start=st, stop=(k == 15),
                                skip_group_check=True)
                for j in units:
                    elementwise(1, j, psums[j], hT[Bp][1][j])

            # projection: y_t[b] = sum_k wproj[:, k] . h2T[k]
            yp = ppool.tile([1, BS], F32, name="yp", tag="yp")
            for k in range(NH):
                nc.tensor.matmul(yp, wproj[:, k : k + 1], hT[Bp][1][k],
                                 start=(k == 0), stop=(k == NH - 1))
            ys = tpool.tile([1, BS], F32, name="ys", tag="ys")
            nc.scalar.copy(ys, yp)
            nc.sync.dma_start(out=y_d[t : t + 1, :], in_=ys)


# ---------------------------------------------------------------------
# Host-side prep + entry point
# ---------------------------------------------------------------------

def _prep_shared(W_ih0, W_hh0, b0, W_ih1, W_hh1, b1, W_proj):
    perm = _gate_perm()
    w0 = np.concatenate([W_ih0, W_hh0], axis=1)[perm, :].T  # [1084, 4096]
    w1 = np.concatenate([W_ih1, W_hh1], axis=1)[perm, :].T  # [2048, 4096]

    def unit_major(w):
        return np.ascontiguousarray(
            w.reshape(w.shape[0], NH, 512).transpose(1, 0, 2))

    w0u = unit_major(w0)
    w1u = unit_major(w1)
    w0x = np.zeros((NH, KXP, 512), np.float32)
    w0x[:, :KX, :] = w0u[:, :KX, :]
    w0h = w0u[:, KX:, :].reshape(NH, NH, 128, 512)
    w1i = w1u[:, :H, :].reshape(NH, NH, 128, 512)
    w1h = w1u[:, H:, :].reshape(NH, NH, 128, 512)
    b0p = b0[perm].reshape(8, 4, 128).transpose(2, 0, 1).reshape(128, 32)
    b1p = b1[perm].reshape(8, 4, 128).transpose(2, 0, 1).reshape(128, 32)
    biasm = np.concatenate([b0p, b1p], axis=1).astype(np.float32)
    wproj = W_proj.reshape(H).reshape(NH, 128).T
    bf = ml_dtypes.bfloat16
    return {
        "w0x": np.ascontiguousarray(w0x).astype(bf),
        "w0h": np.ascontiguousarray(w0h).astype(bf),
        "w1i": np.ascontiguousarray(w1i).astype(bf),
        "w1h": np.ascontiguousarray(w1h).astype(bf),
        "bias": np.ascontiguousarray(biasm),
        "wproj": np.ascontiguousarray(wproj).astype(bf),
    }


def kernel(dec_known, target_y, enc_h, enc_c, last_enc_consumption,
           group_ids, cat_province, cat_customer_type, cat_price_type,
           cat_consumption_level, group_emb, emb_province, emb_customer_type,
           emb_price_type, emb_consumption_level, W_ih0, W_hh0, b0,
           W_ih1, W_hh1, b1, W_proj, b_proj):
    from concourse.bass_utils import run_bass_kernel_spmd

    dec_known = np.asarray(dec_known, np.float32)
    target_y = np.asarray(target_y, np.float32)
    enc_h = np.asarray(enc_h, np.float32)
    enc_c = np.asarray(enc_c, np.float32)
    last_enc_consumption = np.asarray(last_enc_consumption, np.float32)

    # static per-sample features + teacher-forced prev_y -> x [B, T, 60]
    cat_emb = np.concatenate([
        np.asarray(emb_province)[np.asarray(cat_province)],
        np.asarray(emb_customer_type)[np.asarray(cat_customer_type)],
        np.asarray(emb_price_type)[np.asarray(cat_price_type)],
        np.asarray(emb_consumption_level)[np.asarray(cat_consumption_level)],
    ], axis=-1)
    static = np.concatenate([cat_emb, np.asarray(group_emb)[np.asarray(group_ids)]],
                            axis=-1)
    prev_y = np.concatenate([last_enc_consumption[:, None, :],
                             target_y[:, :-1, :]], axis=1)
    x_full = np.concatenate([
        prev_y, dec_known,
        np.broadcast_to(static[:, None, :].astype(np.float32),
                        (B, T, static.shape[-1])),
    ], axis=-1).astype(np.float32)

    shared = _prep_shared(np.asarray(W_ih0, np.float32),
                          np.asarray(W_hh0, np.float32),
                          np.asarray(b0, np.float32),
                          np.asarray(W_ih1, np.float32),
                          np.asarray(W_hh1, np.float32),
                          np.asarray(b1, np.float32),
                          np.asarray(W_proj, np.float32))

    bf = ml_dtypes.bfloat16
    maps = []
    for c in range(NCORES):
        sl = slice(c * BS, (c + 1) * BS)
        xt = np.zeros((T, KXP, BS), np.float32)
        xt[:, :KX, :] = x_full[sl].transpose(1, 2, 0)
        m = dict(shared)
        m["xt"] = xt.astype(bf)
        m["hinit"] = np.ascontiguousarray(
            enc_h[:, sl, :].transpose(0, 2, 1)).astype(bf)
        m["cinit"] = np.ascontiguousarray(
            enc_c[:, sl, :].transpose(0, 2, 1)).astype(np.float32)
        maps.append(m)

    nc = bass.Bass("TRN2", target_bir_lowering=False, debug=False,
                   num_devices=NCORES)
    _emit_kernel(nc)
    import os

    trace = bool(os.environ.get("LSTM_BASS_TRACE"))
    res = run_bass_kernel_spmd(nc, maps, list(range(NCORES)), trace=trace)
    if trace and res.exec_time_ns is not None:
        print(f"HW exec time: {res.exec_time_ns} ns")

    y = np.empty((B, T, 1), np.float32)
    for c in range(NCORES):
        y[c * BS : (c + 1) * BS, :, 0] = res.results[c]["y"].T
    y += np.asarray(b_proj, np.float32)[0]
    return y
